# revision 21
# baseline (speedup 1.0000x reference)
"""Trainium2 Bass kernel for nn_BidirectionalMLP (8-core SPMD).

Math (from the reference, EPS=0.5, BETA=0.5): states stay in [0,1]
after every clipped update, so rho(s)=s; rx = clip(x,0,1) is fixed.
Per relaxation step:
  s1' = clip(0.5*s1 + 0.25*(rx@fw0) + 0.25*(s2@bw1), 0, 1)
  s2' = clip(0.5*s2 + 0.25*(s1@fw1 + s3@bw2), 0, 1)
  s3' = clip(0.5*s3 + 0.5*(s2@fw2), 0, 1)          (free phase)
  s3' = clip(0.5*(s2@fw2) + 0.5*y, 0, 1)           (weak phase)
20 free + 5 weak steps from zero states. Step 1 is degenerate, so the
preamble computes it directly and the loop runs 24 iterations of two
matmul phases (A: s1/s3 update, B: s2 update), phase order alternating
so each AllGather hides behind the opposite phase.

Sharding: hybrid 4-feature x 2-batch. Core c owns batch rows
[128b,128b+128) (b=c//4) and hidden-feature columns [1024f,1024f+1024)
(f=c%4) of s1/s2; s3 ([B,10]) is computed redundantly by the 4 cores
of each batch group. Per step each state is all-gathered in fp8e4m3
over the 4-core feature group only ([[0,1,2,3],[4,5,6,7]]), so a rank
receives 384KB/step instead of the 896KB an 8-way feature shard needs.

Matmuls use fp8 DoubleRow: fw1/bw1/fw2 are pre-scaled by 2^13/2^12,
cast to fp8e4m3 host-side and packed as k-chunk pairs; the gathered
fp8 state pair is the stationary operand, so one PE instruction
contracts 256 rows (numpy sim: end-to-end rel err 1.25e-2 vs the
2e-2 gate). bw2/fw0 stay bf16 (bw2 pre-scaled 2^13 to match the fw1
psum scale). Each phase runs its 16 pair-chunks twice (feature halves
512 wide): the first half's update/transpose/fp8-convert/agin-DMA
overlaps the second half's matmuls, so the AllGather triggers right
after the second half's staging lands. Each of the four 256-wide psum
quarters accumulates in its OWN PSUM bank: two concurrent accumulation
groups in one bank corrupt each other's has_written state (found via
MultiCoreSim: "already a pending group in that zero region").

Measured (neuron-profile, 8 cores): 1053-1079us vs the 1437/1621us
(untraced/traced) baseline; rel err 1.22e-2 (gate 2e-2). Remaining
bottleneck: the per-step AllGather chain (2 x ~11.4us Mesh 4-rank ops
serialized on the single CC stream, pipeline depth of one ~8us phase
per AG), which bounds the iteration at ~42us. A tiny warm-up AllGather
at program start absorbs the one-time collectives init; the first
gathered-chunk DMA is split small so the consumer's first LDWEIGHTS
fires early. Tried and reverted: splitting each phase into 3 sub-loops
(single-matmul loops lose LDWEIGHTS amortization, +105us); keep-warm
dummy matmuls of 12/28 per phase (no gain / +48us).
"""

import os
import numpy as np
import ml_dtypes

import concourse.bass as bass
import concourse.tile as tile
from concourse import bacc, mybir
from concourse.bass_utils import run_bass_kernel_spmd

N_CORES = 8
NF = 4           # feature groups
NB = 2           # batch groups
BL = 128         # batch rows per core
D0 = 1024        # input dim
D = 4096         # hidden dims
D3 = 10          # output dim
D3P = 16         # fw2 pair tile padded width (DR pair step must be 16B-aligned)
F = D // NF      # 1024 features per core per hidden layer
FH = F // 2      # 512  (staging half)
FQ = F // 4      # 256  (psum quarter)
KC0 = D0 // 128  # 8
KC = D // 128    # 32
KP = KC // 2     # 16 k-chunk pairs
N_ITERS = 24     # steps 2..25 (step 1 done in preamble)
FREE_ITERS = 19  # iterations with free-phase s3 update (steps 2..20)
DUMMY_N = 0      # keep-warm matmuls per phase

BF16 = mybir.dt.bfloat16
FP8 = mybir.dt.float8e4
F32 = mybir.dt.float32
OP = mybir.AluOpType
DR = mybir.MatmulPerfMode.DoubleRow
RG = [[0, 1, 2, 3], [4, 5, 6, 7]]

W_SCALE = 8192.0   # fw1/bw1/bw2 pre-scale (power of 2)
F2_SCALE = 4096.0  # fw2 pre-scale
SCL1 = 0.25 / W_SCALE
SCL3 = 0.5 / F2_SCALE

_BUILD_CACHE: dict = {}


def _build(n_iters: int = N_ITERS, free_iters: int = FREE_ITERS,
           dummy_n: int = DUMMY_N):
    dbgenv = os.environ.get("BMLPDBG", "")
    key = (n_iters, free_iters, dummy_n, dbgenv)
    if key in _BUILD_CACHE:
        return _BUILD_CACHE[key]

    nc = bacc.Bacc("TRN2", target_bir_lowering=False, debug=False,
                   num_devices=N_CORES, enable_asserts=False)

    # --- per-core external I/O (weights pre-arranged host-side) ---
    fw0c = nc.dram_tensor("fw0c", [128, KC0 * F], BF16, kind="ExternalInput")
    fw1p = nc.dram_tensor("fw1p", [128, KP * 2 * F], FP8, kind="ExternalInput")
    bw1p = nc.dram_tensor("bw1p", [128, KP * 2 * F], FP8, kind="ExternalInput")
    fw2p = nc.dram_tensor("fw2p", [128, KP * 2 * D3P], FP8, kind="ExternalInput")
    bw2c = nc.dram_tensor("bw2c", [D3, F], BF16, kind="ExternalInput")
    rxT = nc.dram_tensor("rxT", [128, KC0 * BL], BF16, kind="ExternalInput")
    yh = nc.dram_tensor("yh", [D3, BL], F32, kind="ExternalInput")
    o1 = nc.dram_tensor("o1", [BL, F], F32, kind="ExternalOutput")
    o2 = nc.dram_tensor("o2", [BL, F], F32, kind="ExternalOutput")
    o3 = nc.dram_tensor("o3", [D3, BL], F32, kind="ExternalOutput")
    dbg = nc.dram_tensor("dbg", [128, 8], F32, kind="ExternalOutput")

    with tile.TileContext(nc) as tc:
        with tc.tile_pool(name="wp", bufs=1) as wp, \
             tc.tile_pool(name="st", bufs=1) as st, \
             tc.tile_pool(name="wk", bufs=2) as wk, \
             tc.tile_pool(name="gp", bufs=2) as gp, \
             tc.tile_pool(name="pp", bufs=1, space="PSUM") as pp, \
             tc.tile_pool(name="pw", bufs=1, space="PSUM") as pw, \
             tc.tile_pool(name="dp", bufs=2, space="DRAM") as dp:

            # ---- persistent state (batch-major: [128 batch, F feat]) ----
            s1 = st.tile([128, F], BF16)
            s2 = st.tile([128, F], BF16)
            cc_t = st.tile([128, F], F32)   # C = 0.25*(rx@fw0)
            o1f = st.tile([128, F], F32)
            o2f = st.tile([128, F], F32)
            o3f = st.tile([D3, BL], F32)

            # ---- warm-up collective: pays the one-time collectives
            # init during the preamble instead of on the first real AG ----
            wtiny = st.tile([128, 16], FP8)
            nc.vector.memset(wtiny[:], 0.0)
            wcin = dp.tile([128, 16], FP8, tag="wcin", name="wcin")
            nc.sync.dma_start(
                wcin.rearrange("(c p) b -> p c b", p=128),
                wtiny[:].rearrange("p (c b) -> p c b", c=1))
            wcout = dp.tile([128 * NF, 16], FP8, tag="wcout", name="wcout")
            nc.gpsimd.collective_compute(
                "AllGather", OP.bypass, replica_groups=RG,
                ins=[wcin.opt()], outs=[wcout.opt()])

            # ---- weights ----
            w_fw0 = wp.tile([128, KC0 * F], BF16)
            nc.sync.dma_start(w_fw0[:], fw0c[:])
            t_rx = wp.tile([128, KC0 * BL], BF16)
            nc.sync.dma_start(t_rx[:], rxT[:])

            warm = pw.tile([128, 512], F32, tag="warm", name="warm")
            warm_on = [False]

            def keepwarm(n):
                for _ in range(n):
                    nc.tensor.matmul(warm[:], w_fw0[:, 0:128],
                                     w_fw0[:, 0:512],
                                     start=not warm_on[0], stop=True,
                                     skip_group_check=True)
                    warm_on[0] = True

            nc.vector.memset(s2[:], 0.0)
            s3_cur = wk.tile([D3, BL], BF16, tag="s3", name="s3")
            nc.vector.memset(s3_cur[:], 0.0)

            # ---- staging + AllGather helpers ----
            def new_agin(which):
                return dp.tile([KC0 * 128, BL], FP8, tag=f"agin{which}",
                               name=f"agin{which}")

            def stage_range(s_tile, sq, agin, cs, ce):
                """Transpose batch-major s[:, 128cs:128ce] into the
                feature-major fp8 stage tile sq and push that range to
                the AllGather input buffer in DRAM."""
                n = ce - cs
                t_h = wk.tile([128, n, 128], BF16, tag=f"tt{n}",
                              name=f"tt{n}")
                nc.sync.dma_start_transpose(
                    t_h[:], s_tile[:, cs * 128:ce * 128])
                sq3 = sq[:].rearrange("p (c b) -> p c b", b=BL)
                nc.scalar.copy(sq3[:, cs:ce, :], t_h[:])
                nc.sync.dma_start(
                    agin.rearrange("(c p) b -> p c b",
                                   p=128)[:, cs:ce, :],
                    sq3[:, cs:ce, :])

            def stage_half(s_tile, sq, h, agin):
                stage_range(s_tile, sq, agin, 4 * h, 4 * h + 4)

            def launch_ag(which, agin):
                """AllGather over the feature group, DMA gathered
                result back to SBUF."""
                agout = dp.tile([KC0 * 128 * NF, BL], FP8,
                                tag=f"agout{which}", name=f"agout{which}")
                nc.gpsimd.collective_compute(
                    "AllGather", OP.bypass, replica_groups=RG,
                    ins=[agin.opt()], outs=[agout.opt()])
                g = gp.tile([128, KC, BL], FP8, tag=f"g{which}",
                            name=f"g{which}")
                ago = agout.rearrange("(n p) b -> p n b", p=128)
                for c0, c1 in ((0, 2), (2, 12), (12, 22), (22, 32)):
                    nc.sync.dma_start(g[:, c0:c1, :], ago[:, c0:c1, :])
                return g

            def s3_update(p3, s3c, weak, last):
                s3n = o3f if last else wk.tile([D3, BL], BF16, tag="s3",
                                               name="s3")
                u3 = wk.tile([D3, BL], F32, tag="u3", name="u3")
                if weak:
                    nc.vector.scalar_tensor_tensor(
                        u3[:], p3[:], SCL3, t_yh[:], OP.mult, OP.add)
                else:
                    v3 = wk.tile([D3, BL], F32, tag="v3", name="v3")
                    nc.vector.tensor_scalar_mul(v3[:], p3[:], SCL3)
                    nc.vector.scalar_tensor_tensor(
                        u3[:], s3c[:], 0.5, v3[:], OP.mult, OP.add)
                nc.vector.tensor_scalar(s3n[:], u3[:], 0.0, 1.0,
                                        OP.max, OP.min)
                return s3n

            def phase_a(g2, s3c, weak, last, warm_n=None):
                """ps = g2@bw1 (DR), p3 = fw2^T@g2 (DR, feature-major);
                s1,s3 update; AG(s1)."""
                keepwarm(dummy_n if warm_n is None else warm_n)
                h1 = wk.tile([128, F], F32, tag="h1", name="h1")
                nc.vector.scalar_tensor_tensor(h1[:], s1[:], 0.5, cc_t[:],
                                               OP.mult, OP.add)
                ps = [pp.tile([128, FQ], F32, tag=f"q{q}", name=f"pa{q}")[:]
                      for q in range(4)]
                p3t = pp.tile([D3P, BL], F32, tag="p3", name="p3")
                p3 = p3t[0:D3, :]
                wv = w_bw1[:].rearrange("p (j i f) -> p j i f", j=KP, i=2)
                fv = w_fw2[:].rearrange("p (j i f) -> p j i f", j=KP, i=2)
                # fv planes are D3P wide (zero-padded); out partitions 10:16
                # of p3t are unused zeros
                sq = wk.tile([128, KC0 * 128], FP8, tag="sq1", name="sq1")
                agin = None if last else new_agin("1")
                dst = o1f if last else s1
                for gi, qs in enumerate(((0, 1), (2, 3))):
                    for jp in range(KP):
                        lhsT = g2[:, 2 * jp:2 * jp + 2, :]
                        for q in qs:
                            nc.tensor.matmul(
                                ps[q], lhsT,
                                wv[:, jp, :, q * FQ:(q + 1) * FQ],
                                start=(jp == 0), stop=(jp == KP - 1),
                                perf_mode=DR)
                        if gi == 1 and "no_p3" not in dbgenv:
                            nc.tensor.matmul(
                                p3t[:], fv[:, jp], lhsT,
                                start=(jp == 0), stop=(jp == KP - 1),
                                perf_mode=DR)
                    for q in qs:
                        sh = slice(q * FQ, (q + 1) * FQ)
                        u = wk.tile([128, FQ], F32, tag="u", name="u")
                        nc.vector.scalar_tensor_tensor(
                            u[:], ps[q], SCL1, h1[:, sh], OP.mult, OP.add)
                        nc.vector.tensor_scalar(dst[:, sh], u[:], 0.0, 1.0,
                                                OP.max, OP.min)
                        if not last:
                            stage_range(dst, sq, agin, 2 * q, 2 * q + 2)
                if "no_p3" in dbgenv:
                    nc.vector.memset(p3t[:], 0.0)
                s3n = s3_update(p3, s3c, weak, last)
                if last:
                    return None, s3n
                return launch_ag("1", agin), s3n

            def phase_a0():
                """Iteration 0: s2(1)=0, so s1(2)=clip(0.5*s1+C), s3(2)=0."""
                h1 = wk.tile([128, F], F32, tag="h1", name="h1")
                nc.vector.scalar_tensor_tensor(h1[:], s1[:], 0.5, cc_t[:],
                                               OP.mult, OP.add)
                nc.vector.tensor_scalar(s1[:], h1[:], 0.0, 1.0,
                                        OP.max, OP.min)
                s3n = wk.tile([D3, BL], BF16, tag="s3", name="s3")
                nc.vector.memset(s3n[:], 0.0)
                sq = wk.tile([128, KC0 * 128], FP8, tag="sq1", name="sq1")
                agin = new_agin("1")
                stage_half(s1, sq, 0, agin)
                stage_half(s1, sq, 1, agin)
                return launch_ag("1", agin), s3n

            def phase_b(g1, s3c, last, skip_bw2=False, warm_n=None):
                """ps = g1@fw1 (DR) + s3@bw2 (bf16); s2 update; AG(s2)."""
                keepwarm(dummy_n if warm_n is None else warm_n)
                h2 = wk.tile([128, F], F32, tag="h2", name="h2")
                nc.vector.tensor_scalar_mul(h2[:], s2[:], 0.5)
                ps = [pp.tile([128, FQ], F32, tag=f"q{q}", name=f"pb{q}")[:]
                      for q in range(4)]
                wv = w_fw1[:].rearrange("p (j i f) -> p j i f", j=KP, i=2)
                sq = wk.tile([128, KC0 * 128], FP8, tag="sq2", name="sq2")
                agin = None if last else new_agin("2")
                dst = o2f if last else s2
                for qs in ((0, 1), (2, 3)):
                    for jp in range(KP):
                        lhsT = g1[:, 2 * jp:2 * jp + 2, :]
                        for q in qs:
                            nc.tensor.matmul(
                                ps[q], lhsT,
                                wv[:, jp, :, q * FQ:(q + 1) * FQ],
                                start=(jp == 0),
                                stop=(jp == KP - 1 and
                                      (skip_bw2 or "no_bw2" in dbgenv)),
                                perf_mode=DR)
                    if not skip_bw2 and "no_bw2" not in dbgenv:
                        for q in qs:
                            nc.tensor.matmul(
                                ps[q], s3c[:],
                                w_bw2[:, q * FQ:(q + 1) * FQ],
                                start=False, stop=True)
                    for q in qs:
                        sh = slice(q * FQ, (q + 1) * FQ)
                        u = wk.tile([128, FQ], F32, tag="u", name="u")
                        nc.vector.scalar_tensor_tensor(
                            u[:], ps[q], SCL1, h2[:, sh], OP.mult, OP.add)
                        nc.vector.tensor_scalar(dst[:, sh], u[:], 0.0, 1.0,
                                                OP.max, OP.min)
                        if not last:
                            stage_range(dst, sq, agin, 2 * q, 2 * q + 2)
                if last:
                    return None
                return launch_ag("2", agin)

            # ---- preamble: C and step-1 s1, its AG ----
            for h in range(2):
                psc = pw.tile([128, FH], F32, tag="pre", name="psc")
                for k in range(KC0):
                    nc.tensor.matmul(
                        psc[:], t_rx[:, k * BL:(k + 1) * BL],
                        w_fw0[:, k * F + h * FH: k * F + (h + 1) * FH],
                        start=(k == 0), stop=(k == KC0 - 1))
                sh = slice(h * FH, (h + 1) * FH)
                nc.vector.tensor_scalar_mul(cc_t[:, sh], psc[:], 0.25)
                nc.vector.tensor_scalar(s1[:, sh], cc_t[:, sh], 0.0, 1.0,
                                        OP.max, OP.min)
            sq0 = wk.tile([128, KC0 * 128], FP8, tag="sq1", name="sq1")
            agin0 = new_agin("1")
            stage_half(s1, sq0, 0, agin0)
            stage_half(s1, sq0, 1, agin0)
            g1_cur = launch_ag("1", agin0)

            # ---- big weights load (overlaps the first AllGather) ----
            w_fw1 = wp.tile([128, KP * 2 * F], FP8)
            nc.sync.dma_start(w_fw1[:], fw1p[:])
            w_bw1 = wp.tile([128, KP * 2 * F], FP8)
            nc.sync.dma_start(w_bw1[:], bw1p[:])
            w_fw2 = wp.tile([128, KP * 2 * D3P], FP8)
            nc.sync.dma_start(w_fw2[:], fw2p[:])
            w_bw2 = wp.tile([D3, F], BF16)
            nc.sync.dma_start(w_bw2[:], bw2c[:])
            t_yh = wp.tile([D3, BL], F32)
            nc.sync.dma_start(t_yh[:], yh[:])

            g2_cur = None
            for t in range(n_iters):
                weak = t >= free_iters
                last = t == n_iters - 1
                wn = 24 if t < 2 else None
                if t == 0:
                    g1_next, s3_next = phase_a0()
                    g2_next = phase_b(g1_cur, s3_cur, last, skip_bw2=True,
                                      warm_n=wn)
                elif t % 2 == 0:
                    g1_next, s3_next = phase_a(g2_cur, s3_cur, weak, last,
                                               warm_n=wn)
                    g2_next = phase_b(g1_cur, s3_cur, last, warm_n=wn)
                else:
                    g2_next = phase_b(g1_cur, s3_cur, last, warm_n=wn)
                    g1_next, s3_next = phase_a(g2_cur, s3_cur, weak, last,
                                               warm_n=wn)
                g1_cur, g2_cur, s3_cur = g1_next, g2_next, s3_next

            # ---- outputs ----
            nc.sync.dma_start(o1.ap(), o1f[:])
            nc.sync.dma_start(o2.ap(), o2f[:])
            nc.sync.dma_start(o3.ap(), o3f[:])
            dbg_sb = st.tile([128, 8], F32)
            nc.vector.tensor_copy(dbg_sb[:], warm[:, 0:8])
            nc.sync.dma_start(dbg.ap(), dbg_sb[:])

    nc.compile()
    _BUILD_CACHE[key] = nc
    return nc


def _rearr_w(w: np.ndarray, kc: int) -> np.ndarray:
    """[kc*128, M] -> [128, kc*M] with chunk k at cols [k*M,(k+1)*M)."""
    n, m = w.shape
    assert n == kc * 128
    return np.ascontiguousarray(
        w.reshape(kc, 128, m).transpose(1, 0, 2).reshape(128, kc * m))


def _pair_fp8(w: np.ndarray, scale: float) -> np.ndarray:
    """[4096, M] fp32 -> [128, KP*2*M] fp8e4m3 pair layout."""
    n, m = w.shape
    assert n == KP * 2 * 128
    q = np.clip(np.asarray(w, np.float32) * scale, -240.0, 240.0)
    q = q.astype(ml_dtypes.float8_e4m3)
    return np.ascontiguousarray(
        q.reshape(KP, 2, 128, m).transpose(2, 0, 1, 3).reshape(128, KP * 2 * m))


def _prep_in_maps(x, fw0, fw1, fw2, bw1, bw2, y_one_hot):
    bf = ml_dtypes.bfloat16
    x = np.asarray(x, np.float32)
    rx = np.clip(x, 0.0, 1.0)
    fw2_pad = np.zeros((D, D3P), np.float32)
    fw2_pad[:, :D3] = np.asarray(fw2, np.float32)
    fw2_p = _pair_fp8(fw2_pad, F2_SCALE)
    in_maps = []
    for c in range(N_CORES):
        b, f = c // NF, c % NF
        fsl = slice(f * F, (f + 1) * F)
        bsl = slice(b * BL, (b + 1) * BL)
        rxT = np.ascontiguousarray(rx[bsl].T)                  # [1024, 128]
        yh = np.ascontiguousarray(
            0.5 * np.asarray(y_one_hot, np.float32)[bsl].T)    # [10, 128]
        in_maps.append({
            "fw0c": _rearr_w(np.asarray(fw0, np.float32)[:, fsl],
                             KC0).astype(bf),
            "fw1p": _pair_fp8(np.asarray(fw1, np.float32)[:, fsl], W_SCALE),
            "bw1p": _pair_fp8(np.asarray(bw1, np.float32)[:, fsl], W_SCALE),
            "fw2p": fw2_p,
            "bw2c": (np.asarray(bw2, np.float32)[:, fsl] * W_SCALE
                     ).astype(bf),
            "rxT": _rearr_w(rxT, KC0).astype(bf),
            "yh": yh.astype(np.float32),
        })
    return in_maps


def _assemble(results) -> np.ndarray:
    out = np.zeros((NB * BL, 2 * D + D3), np.float32)
    for c in range(N_CORES):
        b, f = c // NF, c % NF
        bsl = slice(b * BL, (b + 1) * BL)
        fsl = slice(f * F, (f + 1) * F)
        out[bsl, fsl] = results[c]["o1"]
        out[bsl, D + f * F:D + (f + 1) * F] = results[c]["o2"]
    for b in range(NB):
        bsl = slice(b * BL, (b + 1) * BL)
        out[bsl, 2 * D:] = results[b * NF]["o3"].T
    return np.ascontiguousarray(out)


def run(inputs: dict, trace: bool = False, n_iters: int = N_ITERS,
        free_iters: int = FREE_ITERS, dummy_n: int = DUMMY_N):
    """Returns (output [256, 8202] fp32, BassKernelResults)."""
    nc = _build(n_iters, free_iters, dummy_n)
    in_maps = _prep_in_maps(
        inputs["x"], inputs["fw0"], inputs["fw1"], inputs["fw2"],
        inputs["bw1"], inputs["bw2"], inputs["y_one_hot"])
    r = run_bass_kernel_spmd(nc, in_maps, core_ids=list(range(N_CORES)),
                             trace=trace)
    return _assemble(r.results), r


def kernel(**inputs) -> np.ndarray:
    out, _ = run(inputs)
    return out
